# revision 10
# baseline (speedup 1.0000x reference)
"""GCNConv on 8 Trainium2 NeuronCores.

out[i] = sum_{(i,j) in E} vals * (x @ W)[j]
x [100000, 256] f32, W [256, 128], edge_row/col [1600000] i32, vals f32.

Design (SPMD over 8 cores, dest-partitioned):
  phase 1: core c computes h_c = x_c @ W for its 12544-row node shard
           (host passes x_c transposed), AllGather -> full h [100352, 128]
           f32 in every core's DRAM.
  phase 2: edges are partitioned by destination (core = dest // 12544) and
           by source quartile (4 tables of 25088 rows so indices fit the
           int16 limit of dma_gather). Per (dest-block of 128, quartile):
           dma_gather fetches message rows h[col] (512B descriptors); a
           one-hot selection matrix S[e, d] = (iota_d == rel_dest[e]) * val[e]
           built on DVE turns the segment sum into a PE matmul accumulated
           in PSUM per dest-block: out_block += S^T @ msgs.
  output: each core writes its [12544, 128] f32 shard; host concatenates.

The instruction stream is identical across cores: per-(block, quartile)
chunk counts are maxed over cores and padded with (idx=0, val=0) edges.
"""

import os

import numpy as np

N_NODES = 100000
N_EDGES = 1600000
IN_F = 256
OUT_F = 128
N_CORES = 8
P = 128
SHARD = 12544  # nodes per core (8 * 12544 = 100352 >= 100000)
N_PAD = SHARD * N_CORES
QUART = N_PAD // 4  # 25088 rows per gather table (< 32768 int16 limit)
N_BLOCKS = SHARD // P  # 98 dest blocks per core
SB = 4  # dest blocks per gather call (superblock)

LAST_EXEC_TIME_NS = None
LAST_RESULT = None


def _enable_profiling():
    """Recreate the missing antenv.axon_hooks NTFF profile hook (container
    image lacks it) so run_bass_kernel_spmd(trace=True) works."""
    import sys
    import types

    if "antenv.axon_hooks" in sys.modules:
        return
    mod = types.ModuleType("antenv.axon_hooks")
    mod._hook = None
    mod.set_axon_ntff_profile_hook = lambda h: setattr(mod, "_hook", h)
    mod.get_axon_ntff_profile_hook = lambda: mod._hook
    sys.modules["antenv.axon_hooks"] = mod
    import antenv

    antenv.axon_hooks = mod
    try:
        from trn_agent_boot.trn_boot import _ntff_profile_via_ctypes

        mod.set_axon_ntff_profile_hook(
            _ntff_profile_via_ctypes("/opt/axon/libaxon_pjrt.so")
        )
    except Exception:
        pass
    import concourse.bass_utils as bu

    bu.upload_artifacts = lambda tmpdir: f"local:{tmpdir}"


def _prep(edge_row, edge_col, edge_vals):
    """Partition + sort edges, compute SPMD-uniform straddle-chunk layout.

    Chunks are 128-edge slices of each (superblock, quartile) cell's
    dest-sorted edge stream. A chunk may straddle one dest-block boundary;
    it always issues two matmuls, against blocks lo and min(lo+1, sb_last),
    with separate one-hot matrices (rel_lo / rel_hi, non-matching edges
    pushed out of iota range).
    """
    core_of = edge_row // SHARD
    N_SB = N_BLOCKS // SB  # superblocks (N_BLOCKS % SB == 0 -> 98/4 no!)
    assert N_BLOCKS % SB == 0 or True
    sbs = [(sb0, min(sb0 + SB, N_BLOCKS)) for sb0 in range(0, N_BLOCKS, SB)]

    percore_sorted = []
    counts = np.zeros((N_CORES, len(sbs), 4), dtype=np.int64)
    for c in range(N_CORES):
        m = core_of == c
        r = edge_row[m]
        col = edge_col[m]
        v = edge_vals[m]
        loc = r - c * SHARD
        blk = loc // P
        rel = loc % P
        q = col // QUART
        lidx = (col - q * QUART).astype(np.int16)
        sbi = blk // SB
        order = np.lexsort((rel, blk, q, sbi))
        blk = blk[order]
        rel = rel[order]
        q = q[order]
        lidx = lidx[order]
        v = v[order]
        sbi = sbi[order]
        key = sbi * 4 + q
        cnt = np.bincount(key, minlength=len(sbs) * 4).reshape(len(sbs), 4)
        counts[c] = cnt
        starts = np.concatenate(([0], np.cumsum(cnt.ravel())))
        percore_sorted.append((starts, blk, rel, lidx, v))

    k_cell = np.maximum(1, -(-counts.max(axis=0) // P))  # [n_sb, 4]
    n_chunks = int(k_cell.sum())

    # global chunk meta + per-core padded arrays
    idx_pads = [np.zeros((n_chunks, P), dtype=np.int16) for _ in range(N_CORES)]
    rlo_pads = [np.full((n_chunks, P), 200.0, dtype=np.float32) for _ in range(N_CORES)]
    rhi_pads = [np.full((n_chunks, P), 200.0, dtype=np.float32) for _ in range(N_CORES)]
    val_pads = [np.zeros((n_chunks, P), dtype=np.float32) for _ in range(N_CORES)]

    calls = []  # list of (quart, [ (chunk_t, lo_block, hi_block), ... ]) per gather call
    sb_flush = []  # per superblock: (calls_range, mm-structure is derived in kernel)
    t = 0
    MAXC = 16  # chunks per gather call (<=2048 idxs)
    for si, (sb0, sb1) in enumerate(sbs):
        sb_calls = []
        for q in range(4):
            k = int(k_cell[si, q])
            # per-core block at each chunk boundary
            lo = np.full(k, 10 ** 9, dtype=np.int64)
            hi = np.full(k, -1, dtype=np.int64)
            for c in range(N_CORES):
                starts, blk, rel, lidx, v = percore_sorted[c]
                s = starts[si * 4 + q]
                n = counts[c, si, q]
                if n == 0:
                    continue
                b_c = blk[s : s + n]
                for ch in range(k):
                    p0 = ch * P
                    p1 = min(p0 + P, n) - 1
                    if p0 < n:
                        lo[ch] = min(lo[ch], b_c[p0])
                        hi[ch] = max(hi[ch], b_c[p1])
            # chunks with no edges on any core: aim at sb0
            for ch in range(k):
                if hi[ch] < 0:
                    lo[ch] = sb0
                    hi[ch] = sb0
            assert np.all(hi - lo <= 1), (hi - lo).max()
            hi = np.minimum(lo + 1, sb1 - 1)

            # fill per-core arrays
            for c in range(N_CORES):
                starts, blk, rel, lidx, v = percore_sorted[c]
                s = starts[si * 4 + q]
                n = counts[c, si, q]
                for ch in range(k):
                    p0 = ch * P
                    p1 = min(p0 + P, n)
                    if p0 >= p1:
                        continue
                    g = t + ch
                    w = p1 - p0
                    bb = blk[s + p0 : s + p1]
                    rr = rel[s + p0 : s + p1]
                    assert np.all((bb == lo[ch]) | (bb == lo[ch] + 1))
                    idx_pads[c][g, :w] = lidx[s + p0 : s + p1]
                    val_pads[c][g, :w] = v[s + p0 : s + p1]
                    mlo = bb == lo[ch]
                    rlo_pads[c][g, :w] = np.where(mlo, rr, 200.0)
                    rhi_pads[c][g, :w] = np.where(~mlo, rr, 200.0)
            # split into calls
            for c0 in range(0, k, MAXC):
                nn = min(MAXC, k - c0)
                chunks = [
                    (t + c0 + j, int(lo[c0 + j]), int(hi[c0 + j]))
                    for j in range(nn)
                ]
                sb_calls.append((q, chunks))
            t += k
        calls.append(sb_calls)
    assert t == n_chunks
    per_core = [
        (idx_pads[c], rlo_pads[c], rhi_pads[c], val_pads[c])
        for c in range(N_CORES)
    ]
    return calls, n_chunks, per_core


def _build_kernel(sb_call_list, n_chunks):
    import concourse.bass as bass
    import concourse.bacc as bacc
    import concourse.tile as tile
    from concourse import mybir

    nc = bacc.Bacc(
        "TRN2", target_bir_lowering=False, debug=False, num_devices=N_CORES
    )
    f32 = mybir.dt.float32
    i16 = mybir.dt.int16

    xT = nc.dram_tensor("xT", [IN_F, SHARD], f32, kind="ExternalInput")
    Wt = nc.dram_tensor("W", [IN_F, OUT_F], f32, kind="ExternalInput")
    bf16 = mybir.dt.bfloat16
    iota_in = nc.dram_tensor("iota", [P, 4 * P], bf16, kind="ExternalInput")
    idx_in = nc.dram_tensor("idx", [P, 8 * n_chunks], i16, kind="ExternalInput")
    rlo_in = nc.dram_tensor("rlo", [P, n_chunks], bf16, kind="ExternalInput")
    rhi_in = nc.dram_tensor("rhi", [P, n_chunks], bf16, kind="ExternalInput")
    val_in = nc.dram_tensor("val", [P, n_chunks], bf16, kind="ExternalInput")
    out_d = nc.dram_tensor("out", [SHARD, OUT_F], f32, kind="ExternalOutput")

    h_c = nc.dram_tensor("h_c", [SHARD, OUT_F], bf16)
    h_full = nc.dram_tensor("h_full", [N_PAD, OUT_F], bf16, addr_space="Shared")

    with tile.TileContext(nc) as tc:
        # ---------------- phase 1: h_c = x_c @ W, AllGather ----------------
        with (
            tc.tile_pool(name="p1x", bufs=1) as p1x,
            tc.tile_pool(name="p1w", bufs=1) as p1w,
            tc.tile_pool(name="p1o", bufs=4) as p1o,
            tc.tile_pool(name="p1ps", bufs=4, space="PSUM") as p1ps,
        ):
            xt_t = p1x.tile([P, 2, SHARD], f32)  # two K-halves
            nc.sync.dma_start(
                xt_t[:], xT[:].rearrange("(k p) s -> p k s", p=P)
            )
            w_t = p1w.tile([P, 2, OUT_F], f32)
            nc.sync.dma_start(w_t[:], Wt[:].rearrange("(k p) f -> p k f", p=P))
            for t in range(N_BLOCKS):
                ps = p1ps.tile([P, OUT_F], f32, space="PSUM")
                for k in range(2):
                    nc.tensor.matmul(
                        ps[:],
                        lhsT=xt_t[:, k, t * P : (t + 1) * P],
                        rhs=w_t[:, k, :],
                        start=(k == 0),
                        stop=(k == 1),
                    )
                ho = p1o.tile([P, OUT_F], bf16)
                nc.scalar.copy(ho[:], ps[:])
                nc.sync.dma_start(out=h_c[t * P : (t + 1) * P, :], in_=ho[:])
            nc.gpsimd.collective_compute(
                "AllGather",
                mybir.AluOpType.bypass,
                replica_groups=[list(range(N_CORES))],
                ins=[h_c[:]],
                outs=[h_full[:]],
            )

        # ---------------- phase 2: gather + one-hot matmul segsum ---------
        with (
            tc.tile_pool(name="meta", bufs=1) as meta,
            tc.tile_pool(name="msgs", bufs=8) as msgs_p,
            tc.tile_pool(name="sel", bufs=40) as sel_p,
            tc.tile_pool(name="outs", bufs=8) as outs_p,
            tc.tile_pool(name="ps", bufs=8, space="PSUM") as ps_p,
        ):
            iota_t = meta.tile([P, 4, P], bf16)
            nc.sync.dma_start(
                iota_t[:], iota_in[:].rearrange("p (g d) -> p g d", g=4)
            )
            idx_t = meta.tile([P, 8 * n_chunks], i16)
            nc.sync.dma_start(idx_t[:], idx_in[:])
            rlo_t = meta.tile([P, n_chunks], bf16)
            nc.sync.dma_start(rlo_t[:], rlo_in[:])
            rhi_t = meta.tile([P, n_chunks], bf16)
            nc.sync.dma_start(rhi_t[:], rhi_in[:])
            val_t = meta.tile([P, n_chunks], bf16)
            nc.sync.dma_start(val_t[:], val_in[:])

            for sb_calls in sb_call_list:
                psum_of_block = {}
                for q, chunks in sb_calls:
                    ncall = len(chunks)
                    ni = ncall * P
                    c0 = chunks[0][0]
                    m = msgs_p.tile([P, ncall, OUT_F], bf16, tag="m", name="m")
                    nc.gpsimd.dma_gather(
                        m[:],
                        h_full[q * QUART : (q + 1) * QUART, :],
                        idx_t[:, c0 * 8 : (c0 + ncall) * 8],
                        ni,
                        ni,
                        OUT_F,
                        single_packet=False,
                    )
                    s_lo = {}
                    s_hi = {}
                    for g0 in range(0, ncall, 4):
                        g = min(4, ncall - g0)
                        s4l = sel_p.tile([P, 4, P], bf16, tag="s4", name="s4l")
                        nc.vector.tensor_tensor(
                            out=s4l[:, :g, :],
                            in0=iota_t[:, :g, :],
                            in1=rlo_t[:, c0 + g0 : c0 + g0 + g, None].to_broadcast([P, g, P]),
                            op=mybir.AluOpType.is_equal,
                        )
                        nc.vector.tensor_tensor(
                            out=s4l[:, :g, :],
                            in0=s4l[:, :g, :],
                            in1=val_t[:, c0 + g0 : c0 + g0 + g, None].to_broadcast([P, g, P]),
                            op=mybir.AluOpType.mult,
                        )
                        s4h = sel_p.tile([P, 4, P], bf16, tag="s4", name="s4h")
                        nc.vector.tensor_tensor(
                            out=s4h[:, :g, :],
                            in0=iota_t[:, :g, :],
                            in1=rhi_t[:, c0 + g0 : c0 + g0 + g, None].to_broadcast([P, g, P]),
                            op=mybir.AluOpType.is_equal,
                        )
                        nc.vector.tensor_tensor(
                            out=s4h[:, :g, :],
                            in0=s4h[:, :g, :],
                            in1=val_t[:, c0 + g0 : c0 + g0 + g, None].to_broadcast([P, g, P]),
                            op=mybir.AluOpType.mult,
                        )
                        for jj in range(g):
                            s_lo[g0 + jj] = (s4l, jj)
                            s_hi[g0 + jj] = (s4h, jj)
                    for j, (ch, blo, bhi) in enumerate(chunks):
                        for b, stab in ((blo, s_lo), (bhi, s_hi)):
                            if b not in psum_of_block:
                                psum_of_block[b] = (
                                    ps_p.tile([P, OUT_F], f32, space="PSUM", tag="psb", name="psb"),
                                    [],
                                )
                            ps, plist = psum_of_block[b]
                            s4, jj = stab[j]
                            plist.append((s4, jj, m, j))
                for b, (ps, plist) in sorted(psum_of_block.items()):
                    for n, (s4, jj, mm, j) in enumerate(plist):
                        nc.tensor.matmul(
                            ps[:],
                            lhsT=s4[:, jj, :],
                            rhs=mm[:, j, :],
                            start=(n == 0),
                            stop=(n == len(plist) - 1),
                        )
                    ot = outs_p.tile([P, OUT_F], f32)
                    nc.scalar.copy(ot[:], ps[:])
                    nc.sync.dma_start(
                        out=out_d[b * P : (b + 1) * P, :], in_=ot[:]
                    )

    nc.compile()
    return nc


def kernel(x, weight, edge_row, edge_col, edge_vals):
    x = np.ascontiguousarray(x, dtype=np.float32)
    weight = np.ascontiguousarray(weight, dtype=np.float32)
    edge_row = np.asarray(edge_row, dtype=np.int64)
    edge_col = np.asarray(edge_col, dtype=np.int64)
    edge_vals = np.asarray(edge_vals, dtype=np.float32)

    trace = os.environ.get("GCN_TRACE", "0") == "1"
    if trace:
        _enable_profiling()

    from concourse.bass_utils import run_bass_kernel_spmd

    sb_call_list, n_chunks, per_core = _prep(edge_row, edge_col, edge_vals)
    nc = _build_kernel(sb_call_list, n_chunks)

    # host-side input staging
    x_pad = np.zeros((N_PAD, IN_F), dtype=np.float32)
    x_pad[:N_NODES] = x
    import ml_dtypes

    iota = np.broadcast_to(
        np.tile(np.arange(P, dtype=np.float32), 4)[None, :], (P, 4 * P)
    ).astype(ml_dtypes.bfloat16)

    in_maps = []
    flat_calls = [cq for sb_calls in sb_call_list for cq in sb_calls]
    for c in range(N_CORES):
        idx_pad, rlo_pad, rhi_pad, val_pad = per_core[c]
        # wrap indices per call: within a call of n chunks (= n*128 idxs),
        # idx i -> partition i%16, col i//16, replicated to 128 partitions.
        idx_w = np.zeros((P, 8 * n_chunks), dtype=np.int16)
        for q, chunks in flat_calls:
            c0 = chunks[0][0]
            n = len(chunks)
            flat = idx_pad[c0 : c0 + n].reshape(n * P)
            w = flat.reshape(n * 8, 16).T  # [16, n*8]
            idx_w[:, c0 * 8 : (c0 + n) * 8] = np.tile(w, (8, 1))
        in_maps.append(
            {
                "xT": np.ascontiguousarray(
                    x_pad[c * SHARD : (c + 1) * SHARD].T
                ),
                "W": weight,
                "iota": iota,
                "idx": idx_w,
                "rlo": np.ascontiguousarray(rlo_pad.T).astype(
                    ml_dtypes.bfloat16
                ),
                "rhi": np.ascontiguousarray(rhi_pad.T).astype(
                    ml_dtypes.bfloat16
                ),
                "val": np.ascontiguousarray(val_pad.T).astype(
                    ml_dtypes.bfloat16
                ),
            }
        )

    res = run_bass_kernel_spmd(
        nc,
        in_maps,
        list(range(N_CORES)),
        trace=trace,
        trace_cores=[0] if trace else None,
    )
    global LAST_EXEC_TIME_NS, LAST_RESULT
    LAST_EXEC_TIME_NS = res.exec_time_ns
    LAST_RESULT = res

    out = np.empty((N_NODES, OUT_F), dtype=np.float32)
    for c in range(N_CORES):
        lo = c * SHARD
        hi = min((c + 1) * SHARD, N_NODES)
        if hi > lo:
            out[lo:hi] = res.results[c]["out"][: hi - lo]
    return out


# revision 11
# speedup vs baseline: 1.1701x; 1.1701x over previous
"""GCNConv on 8 Trainium2 NeuronCores.

out[i] = sum_{(i,j) in E} vals * (x @ W)[j]
x [100000, 256] f32, W [256, 128], edge_row/col [1600000] i32, vals f32.

Design (SPMD over 8 cores, dest-partitioned):
  phase 1: core c computes h_c = x_c @ W for its 12544-row node shard
           (host passes x_c transposed), AllGather -> full h [100352, 128]
           f32 in every core's DRAM.
  phase 2: edges are partitioned by destination (core = dest // 12544) and
           by source quartile (4 tables of 25088 rows so indices fit the
           int16 limit of dma_gather). Per (dest-block of 128, quartile):
           dma_gather fetches message rows h[col] (512B descriptors); a
           one-hot selection matrix S[e, d] = (iota_d == rel_dest[e]) * val[e]
           built on DVE turns the segment sum into a PE matmul accumulated
           in PSUM per dest-block: out_block += S^T @ msgs.
  output: each core writes its [12544, 128] f32 shard; host concatenates.

The instruction stream is identical across cores: per-(block, quartile)
chunk counts are maxed over cores and padded with (idx=0, val=0) edges.
"""

import os

import numpy as np

N_NODES = 100000
N_EDGES = 1600000
IN_F = 256
OUT_F = 128
N_CORES = 8
P = 128
SHARD = 12544  # nodes per core (8 * 12544 = 100352 >= 100000)
N_PAD = SHARD * N_CORES
QUART = N_PAD // 4  # 25088 rows per gather table (< 32768 int16 limit)
N_BLOCKS = SHARD // P  # 98 dest blocks per core
SB = 4  # dest blocks per gather call (superblock)

LAST_EXEC_TIME_NS = None
LAST_RESULT = None


def _enable_profiling():
    """Recreate the missing antenv.axon_hooks NTFF profile hook (container
    image lacks it) so run_bass_kernel_spmd(trace=True) works."""
    import sys
    import types

    if "antenv.axon_hooks" in sys.modules:
        return
    mod = types.ModuleType("antenv.axon_hooks")
    mod._hook = None
    mod.set_axon_ntff_profile_hook = lambda h: setattr(mod, "_hook", h)
    mod.get_axon_ntff_profile_hook = lambda: mod._hook
    sys.modules["antenv.axon_hooks"] = mod
    import antenv

    antenv.axon_hooks = mod
    try:
        from trn_agent_boot.trn_boot import _ntff_profile_via_ctypes

        mod.set_axon_ntff_profile_hook(
            _ntff_profile_via_ctypes("/opt/axon/libaxon_pjrt.so")
        )
    except Exception:
        pass
    import concourse.bass_utils as bu

    bu.upload_artifacts = lambda tmpdir: f"local:{tmpdir}"


def _prep(edge_row, edge_col, edge_vals):
    """Partition + sort edges, compute SPMD-uniform straddle-chunk layout.

    Chunks are 128-edge slices of each (superblock, quartile) cell's
    dest-sorted edge stream. A chunk may straddle one dest-block boundary;
    it always issues two matmuls, against blocks lo and min(lo+1, sb_last),
    with separate one-hot matrices (rel_lo / rel_hi, non-matching edges
    pushed out of iota range).
    """
    core_of = edge_row // SHARD
    N_SB = N_BLOCKS // SB  # superblocks (N_BLOCKS % SB == 0 -> 98/4 no!)
    assert N_BLOCKS % SB == 0 or True
    sbs = [(sb0, min(sb0 + SB, N_BLOCKS)) for sb0 in range(0, N_BLOCKS, SB)]

    percore_sorted = []
    counts = np.zeros((N_CORES, len(sbs), 4), dtype=np.int64)
    for c in range(N_CORES):
        m = core_of == c
        r = edge_row[m]
        col = edge_col[m]
        v = edge_vals[m]
        loc = r - c * SHARD
        blk = loc // P
        rel = loc % P
        q = col // QUART
        lidx = (col - q * QUART).astype(np.int16)
        sbi = blk // SB
        order = np.lexsort((rel, blk, q, sbi))
        blk = blk[order]
        rel = rel[order]
        q = q[order]
        lidx = lidx[order]
        v = v[order]
        sbi = sbi[order]
        key = sbi * 4 + q
        cnt = np.bincount(key, minlength=len(sbs) * 4).reshape(len(sbs), 4)
        counts[c] = cnt
        starts = np.concatenate(([0], np.cumsum(cnt.ravel())))
        percore_sorted.append((starts, blk, rel, lidx, v))

    k_cell = np.maximum(1, -(-counts.max(axis=0) // P))  # [n_sb, 4]
    n_chunks = int(k_cell.sum())

    # global chunk meta + per-core padded arrays
    idx_pads = [np.zeros((n_chunks, P), dtype=np.int16) for _ in range(N_CORES)]
    rlo_pads = [np.full((n_chunks, P), 200.0, dtype=np.float32) for _ in range(N_CORES)]
    rhi_pads = [np.full((n_chunks, P), 200.0, dtype=np.float32) for _ in range(N_CORES)]
    val_pads = [np.zeros((n_chunks, P), dtype=np.float32) for _ in range(N_CORES)]

    calls = []  # list of (quart, [ (chunk_t, lo_block, hi_block), ... ]) per gather call
    sb_flush = []  # per superblock: (calls_range, mm-structure is derived in kernel)
    t = 0
    MAXC = 16  # chunks per gather call (<=2048 idxs)
    for si, (sb0, sb1) in enumerate(sbs):
        sb_calls = []
        for q in range(4):
            k = int(k_cell[si, q])
            # per-core block at each chunk boundary
            lo = np.full(k, 10 ** 9, dtype=np.int64)
            hi = np.full(k, -1, dtype=np.int64)
            for c in range(N_CORES):
                starts, blk, rel, lidx, v = percore_sorted[c]
                s = starts[si * 4 + q]
                n = counts[c, si, q]
                if n == 0:
                    continue
                b_c = blk[s : s + n]
                for ch in range(k):
                    p0 = ch * P
                    p1 = min(p0 + P, n) - 1
                    if p0 < n:
                        lo[ch] = min(lo[ch], b_c[p0])
                        hi[ch] = max(hi[ch], b_c[p1])
            # chunks with no edges on any core: aim at sb0
            for ch in range(k):
                if hi[ch] < 0:
                    lo[ch] = sb0
                    hi[ch] = sb0
            assert np.all(hi - lo <= 1), (hi - lo).max()

            # fill per-core arrays
            for c in range(N_CORES):
                starts, blk, rel, lidx, v = percore_sorted[c]
                s = starts[si * 4 + q]
                n = counts[c, si, q]
                for ch in range(k):
                    p0 = ch * P
                    p1 = min(p0 + P, n)
                    if p0 >= p1:
                        continue
                    g = t + ch
                    w = p1 - p0
                    bb = blk[s + p0 : s + p1]
                    rr = rel[s + p0 : s + p1]
                    assert np.all((bb == lo[ch]) | (bb == lo[ch] + 1))
                    idx_pads[c][g, :w] = lidx[s + p0 : s + p1]
                    val_pads[c][g, :w] = v[s + p0 : s + p1]
                    mlo = bb == lo[ch]
                    rlo_pads[c][g, :w] = np.where(mlo, rr, 200.0)
                    rhi_pads[c][g, :w] = np.where(~mlo, rr, 200.0)
            # split into calls
            for c0 in range(0, k, MAXC):
                nn = min(MAXC, k - c0)
                chunks = [
                    (t + c0 + j, int(lo[c0 + j]), int(hi[c0 + j]))
                    for j in range(nn)
                ]
                sb_calls.append((q, chunks))
            t += k
        calls.append(sb_calls)
    assert t == n_chunks
    per_core = [
        (idx_pads[c], rlo_pads[c], rhi_pads[c], val_pads[c])
        for c in range(N_CORES)
    ]
    return calls, n_chunks, per_core


def _build_kernel(sb_call_list, n_chunks):
    import concourse.bass as bass
    import concourse.bacc as bacc
    import concourse.tile as tile
    from concourse import mybir

    nc = bacc.Bacc(
        "TRN2", target_bir_lowering=False, debug=False, num_devices=N_CORES
    )
    f32 = mybir.dt.float32
    i16 = mybir.dt.int16

    xT = nc.dram_tensor("xT", [IN_F, SHARD], f32, kind="ExternalInput")
    Wt = nc.dram_tensor("W", [IN_F, OUT_F], f32, kind="ExternalInput")
    bf16 = mybir.dt.bfloat16
    iota_in = nc.dram_tensor("iota", [P, 4 * P], bf16, kind="ExternalInput")
    idx_in = nc.dram_tensor("idx", [P, 8 * n_chunks], i16, kind="ExternalInput")
    rlo_in = nc.dram_tensor("rlo", [P, n_chunks], bf16, kind="ExternalInput")
    rhi_in = nc.dram_tensor("rhi", [P, n_chunks], bf16, kind="ExternalInput")
    val_in = nc.dram_tensor("val", [P, n_chunks], bf16, kind="ExternalInput")
    out_d = nc.dram_tensor("out", [SHARD, OUT_F], f32, kind="ExternalOutput")

    h_c = nc.dram_tensor("h_c", [SHARD, OUT_F], bf16)
    h_full = nc.dram_tensor("h_full", [N_PAD, OUT_F], bf16, addr_space="Shared")

    with tile.TileContext(nc) as tc:
        # ---------------- phase 1: h_c = x_c @ W, AllGather ----------------
        with (
            tc.tile_pool(name="p1x", bufs=1) as p1x,
            tc.tile_pool(name="p1w", bufs=1) as p1w,
            tc.tile_pool(name="p1o", bufs=4) as p1o,
            tc.tile_pool(name="p1ps", bufs=4, space="PSUM") as p1ps,
        ):
            xt_t = p1x.tile([P, 2, SHARD], f32)  # two K-halves
            nc.sync.dma_start(
                xt_t[:], xT[:].rearrange("(k p) s -> p k s", p=P)
            )
            w_t = p1w.tile([P, 2, OUT_F], f32)
            nc.sync.dma_start(w_t[:], Wt[:].rearrange("(k p) f -> p k f", p=P))
            for t in range(N_BLOCKS):
                ps = p1ps.tile([P, OUT_F], f32, space="PSUM")
                for k in range(2):
                    nc.tensor.matmul(
                        ps[:],
                        lhsT=xt_t[:, k, t * P : (t + 1) * P],
                        rhs=w_t[:, k, :],
                        start=(k == 0),
                        stop=(k == 1),
                    )
                ho = p1o.tile([P, OUT_F], bf16)
                nc.scalar.copy(ho[:], ps[:])
                nc.sync.dma_start(out=h_c[t * P : (t + 1) * P, :], in_=ho[:])
            nc.gpsimd.collective_compute(
                "AllGather",
                mybir.AluOpType.bypass,
                replica_groups=[list(range(N_CORES))],
                ins=[h_c[:]],
                outs=[h_full[:]],
            )

        # ---------------- phase 2: gather + one-hot matmul segsum ---------
        with (
            tc.tile_pool(name="meta", bufs=1) as meta,
            tc.tile_pool(name="msgs", bufs=8) as msgs_p,
            tc.tile_pool(name="sel", bufs=40) as sel_p,
            tc.tile_pool(name="outs", bufs=8) as outs_p,
            tc.tile_pool(name="ps", bufs=8, space="PSUM") as ps_p,
        ):
            iota_t = meta.tile([P, 4, P], bf16)
            nc.sync.dma_start(
                iota_t[:], iota_in[:].rearrange("p (g d) -> p g d", g=4)
            )
            idx_t = meta.tile([P, 8 * n_chunks], i16)
            nc.sync.dma_start(idx_t[:], idx_in[:])
            rlo_t = meta.tile([P, n_chunks], bf16)
            nc.sync.dma_start(rlo_t[:], rlo_in[:])
            rhi_t = meta.tile([P, n_chunks], bf16)
            nc.sync.dma_start(rhi_t[:], rhi_in[:])
            val_t = meta.tile([P, n_chunks], bf16)
            nc.sync.dma_start(val_t[:], val_in[:])

            for sb_calls in sb_call_list:
                psum_of_block = {}
                for q, chunks in sb_calls:
                    ncall = len(chunks)
                    ni = ncall * P
                    c0 = chunks[0][0]
                    m = msgs_p.tile([P, ncall, OUT_F], bf16, tag="m", name="m")
                    nc.gpsimd.dma_gather(
                        m[:],
                        h_full[q * QUART : (q + 1) * QUART, :],
                        idx_t[:, c0 * 8 : (c0 + ncall) * 8],
                        ni,
                        ni,
                        OUT_F,
                        single_packet=False,
                    )
                    s_lo = {}
                    s_hi = {}
                    for g0 in range(0, ncall, 4):
                        g = min(4, ncall - g0)
                        s4l = sel_p.tile([P, 4, P], bf16, tag="s4", name="s4l")
                        nc.vector.tensor_tensor(
                            out=s4l[:, :g, :],
                            in0=iota_t[:, :g, :],
                            in1=rlo_t[:, c0 + g0 : c0 + g0 + g, None].to_broadcast([P, g, P]),
                            op=mybir.AluOpType.is_equal,
                        )
                        nc.vector.tensor_tensor(
                            out=s4l[:, :g, :],
                            in0=s4l[:, :g, :],
                            in1=val_t[:, c0 + g0 : c0 + g0 + g, None].to_broadcast([P, g, P]),
                            op=mybir.AluOpType.mult,
                        )
                        straddle = any(
                            chunks[g0 + jj][2] != chunks[g0 + jj][1]
                            for jj in range(g)
                        )
                        s4h = None
                        if straddle:
                            s4h = sel_p.tile([P, 4, P], bf16, tag="s4", name="s4h")
                            nc.vector.tensor_tensor(
                                out=s4h[:, :g, :],
                                in0=iota_t[:, :g, :],
                                in1=rhi_t[:, c0 + g0 : c0 + g0 + g, None].to_broadcast([P, g, P]),
                                op=mybir.AluOpType.is_equal,
                            )
                            nc.vector.tensor_tensor(
                                out=s4h[:, :g, :],
                                in0=s4h[:, :g, :],
                                in1=val_t[:, c0 + g0 : c0 + g0 + g, None].to_broadcast([P, g, P]),
                                op=mybir.AluOpType.mult,
                            )
                        for jj in range(g):
                            s_lo[g0 + jj] = (s4l, jj)
                            s_hi[g0 + jj] = (s4h, jj)
                    for j, (ch, blo, bhi) in enumerate(chunks):
                        targets = [(blo, s_lo)]
                        if bhi != blo:
                            targets.append((bhi, s_hi))
                        for b, stab in targets:
                            if b not in psum_of_block:
                                psum_of_block[b] = (
                                    ps_p.tile([P, OUT_F], f32, space="PSUM", tag="psb", name="psb"),
                                    [],
                                )
                            ps, plist = psum_of_block[b]
                            s4, jj = stab[j]
                            plist.append((s4, jj, m, j))
                for b, (ps, plist) in sorted(psum_of_block.items()):
                    for n, (s4, jj, mm, j) in enumerate(plist):
                        nc.tensor.matmul(
                            ps[:],
                            lhsT=s4[:, jj, :],
                            rhs=mm[:, j, :],
                            start=(n == 0),
                            stop=(n == len(plist) - 1),
                        )
                    ot = outs_p.tile([P, OUT_F], f32)
                    nc.scalar.copy(ot[:], ps[:])
                    nc.sync.dma_start(
                        out=out_d[b * P : (b + 1) * P, :], in_=ot[:]
                    )

    nc.compile()
    return nc


def kernel(x, weight, edge_row, edge_col, edge_vals):
    x = np.ascontiguousarray(x, dtype=np.float32)
    weight = np.ascontiguousarray(weight, dtype=np.float32)
    edge_row = np.asarray(edge_row, dtype=np.int64)
    edge_col = np.asarray(edge_col, dtype=np.int64)
    edge_vals = np.asarray(edge_vals, dtype=np.float32)

    trace = os.environ.get("GCN_TRACE", "0") == "1"
    if trace:
        _enable_profiling()

    from concourse.bass_utils import run_bass_kernel_spmd

    sb_call_list, n_chunks, per_core = _prep(edge_row, edge_col, edge_vals)
    nc = _build_kernel(sb_call_list, n_chunks)

    # host-side input staging
    x_pad = np.zeros((N_PAD, IN_F), dtype=np.float32)
    x_pad[:N_NODES] = x
    import ml_dtypes

    iota = np.broadcast_to(
        np.tile(np.arange(P, dtype=np.float32), 4)[None, :], (P, 4 * P)
    ).astype(ml_dtypes.bfloat16)

    in_maps = []
    flat_calls = [cq for sb_calls in sb_call_list for cq in sb_calls]
    for c in range(N_CORES):
        idx_pad, rlo_pad, rhi_pad, val_pad = per_core[c]
        # wrap indices per call: within a call of n chunks (= n*128 idxs),
        # idx i -> partition i%16, col i//16, replicated to 128 partitions.
        idx_w = np.zeros((P, 8 * n_chunks), dtype=np.int16)
        for q, chunks in flat_calls:
            c0 = chunks[0][0]
            n = len(chunks)
            flat = idx_pad[c0 : c0 + n].reshape(n * P)
            w = flat.reshape(n * 8, 16).T  # [16, n*8]
            idx_w[:, c0 * 8 : (c0 + n) * 8] = np.tile(w, (8, 1))
        in_maps.append(
            {
                "xT": np.ascontiguousarray(
                    x_pad[c * SHARD : (c + 1) * SHARD].T
                ),
                "W": weight,
                "iota": iota,
                "idx": idx_w,
                "rlo": np.ascontiguousarray(rlo_pad.T).astype(
                    ml_dtypes.bfloat16
                ),
                "rhi": np.ascontiguousarray(rhi_pad.T).astype(
                    ml_dtypes.bfloat16
                ),
                "val": np.ascontiguousarray(val_pad.T).astype(
                    ml_dtypes.bfloat16
                ),
            }
        )

    res = run_bass_kernel_spmd(
        nc,
        in_maps,
        list(range(N_CORES)),
        trace=trace,
        trace_cores=[0] if trace else None,
    )
    global LAST_EXEC_TIME_NS, LAST_RESULT
    LAST_EXEC_TIME_NS = res.exec_time_ns
    LAST_RESULT = res

    out = np.empty((N_NODES, OUT_F), dtype=np.float32)
    for c in range(N_CORES):
        lo = c * SHARD
        hi = min((c + 1) * SHARD, N_NODES)
        if hi > lo:
            out[lo:hi] = res.results[c]["out"][: hi - lo]
    return out


# revision 12
# speedup vs baseline: 1.3471x; 1.1513x over previous
"""GCNConv on 8 Trainium2 NeuronCores.

out[i] = sum_{(i,j) in E} vals * (x @ W)[j]
x [100000, 256] f32, W [256, 128], edge_row/col [1600000] i32, vals f32.

Design (SPMD over 8 cores, dest-partitioned):
  phase 1: core c computes h_c = x_c @ W for its 12544-row node shard
           (host passes x_c transposed), AllGather -> full h [100352, 128]
           f32 in every core's DRAM.
  phase 2: edges are partitioned by destination (core = dest // 12544) and
           by source quartile (4 tables of 25088 rows so indices fit the
           int16 limit of dma_gather). Per (dest-block of 128, quartile):
           dma_gather fetches message rows h[col] (512B descriptors); a
           one-hot selection matrix S[e, d] = (iota_d == rel_dest[e]) * val[e]
           built on DVE turns the segment sum into a PE matmul accumulated
           in PSUM per dest-block: out_block += S^T @ msgs.
  output: each core writes its [12544, 128] f32 shard; host concatenates.

The instruction stream is identical across cores: per-(block, quartile)
chunk counts are maxed over cores and padded with (idx=0, val=0) edges.
"""

import os

import numpy as np

N_NODES = 100000
N_EDGES = 1600000
IN_F = 256
OUT_F = 128
N_CORES = 8
P = 128
SHARD = 12544  # nodes per core (8 * 12544 = 100352 >= 100000)
N_PAD = SHARD * N_CORES
QUART = N_PAD // 4  # 25088 rows per gather table (< 32768 int16 limit)
N_BLOCKS = SHARD // P  # 98 dest blocks per core
SB = 4  # dest blocks per gather call (superblock)

LAST_EXEC_TIME_NS = None
LAST_RESULT = None


def _enable_profiling():
    """Recreate the missing antenv.axon_hooks NTFF profile hook (container
    image lacks it) so run_bass_kernel_spmd(trace=True) works."""
    import sys
    import types

    if "antenv.axon_hooks" in sys.modules:
        return
    mod = types.ModuleType("antenv.axon_hooks")
    mod._hook = None
    mod.set_axon_ntff_profile_hook = lambda h: setattr(mod, "_hook", h)
    mod.get_axon_ntff_profile_hook = lambda: mod._hook
    sys.modules["antenv.axon_hooks"] = mod
    import antenv

    antenv.axon_hooks = mod
    try:
        from trn_agent_boot.trn_boot import _ntff_profile_via_ctypes

        mod.set_axon_ntff_profile_hook(
            _ntff_profile_via_ctypes("/opt/axon/libaxon_pjrt.so")
        )
    except Exception:
        pass
    import concourse.bass_utils as bu

    bu.upload_artifacts = lambda tmpdir: f"local:{tmpdir}"


def _prep(edge_row, edge_col, edge_vals):
    """Partition + sort edges, compute SPMD-uniform straddle-chunk layout.

    Chunks are 128-edge slices of each (superblock, quartile) cell's
    dest-sorted edge stream. A chunk may straddle one dest-block boundary;
    it always issues two matmuls, against blocks lo and min(lo+1, sb_last),
    with separate one-hot matrices (rel_lo / rel_hi, non-matching edges
    pushed out of iota range).
    """
    core_of = edge_row // SHARD
    N_SB = N_BLOCKS // SB  # superblocks (N_BLOCKS % SB == 0 -> 98/4 no!)
    assert N_BLOCKS % SB == 0 or True
    sbs = [(sb0, min(sb0 + SB, N_BLOCKS)) for sb0 in range(0, N_BLOCKS, SB)]

    percore_sorted = []
    counts = np.zeros((N_CORES, len(sbs), 4), dtype=np.int64)
    for c in range(N_CORES):
        m = core_of == c
        r = edge_row[m]
        col = edge_col[m]
        v = edge_vals[m]
        loc = r - c * SHARD
        blk = loc // P
        rel = loc % P
        q = col // QUART
        lidx = (col - q * QUART).astype(np.int16)
        sbi = blk // SB
        order = np.lexsort((rel, blk, q, sbi))
        blk = blk[order]
        rel = rel[order]
        q = q[order]
        lidx = lidx[order]
        v = v[order]
        sbi = sbi[order]
        key = sbi * 4 + q
        cnt = np.bincount(key, minlength=len(sbs) * 4).reshape(len(sbs), 4)
        counts[c] = cnt
        starts = np.concatenate(([0], np.cumsum(cnt.ravel())))
        percore_sorted.append((starts, blk, rel, lidx, v))

    k_cell = np.maximum(1, -(-counts.max(axis=0) // P))  # [n_sb, 4]
    n_chunks = int(k_cell.sum())

    # global chunk meta + per-core padded arrays
    idx_pads = [np.zeros((n_chunks, P), dtype=np.int16) for _ in range(N_CORES)]
    rlo_pads = [np.full((n_chunks, P), 200.0, dtype=np.float32) for _ in range(N_CORES)]
    rhi_pads = [np.full((n_chunks, P), 200.0, dtype=np.float32) for _ in range(N_CORES)]
    val_pads = [np.zeros((n_chunks, P), dtype=np.float32) for _ in range(N_CORES)]

    calls = []  # list of (quart, [ (chunk_t, lo_block, hi_block), ... ]) per gather call
    sb_flush = []  # per superblock: (calls_range, mm-structure is derived in kernel)
    t = 0
    MAXC = 16  # chunks per gather call (<=2048 idxs)
    for si, (sb0, sb1) in enumerate(sbs):
        sb_calls = []
        for q in range(4):
            k = int(k_cell[si, q])
            # per-core block at each chunk boundary
            lo = np.full(k, 10 ** 9, dtype=np.int64)
            hi = np.full(k, -1, dtype=np.int64)
            for c in range(N_CORES):
                starts, blk, rel, lidx, v = percore_sorted[c]
                s = starts[si * 4 + q]
                n = counts[c, si, q]
                if n == 0:
                    continue
                b_c = blk[s : s + n]
                for ch in range(k):
                    p0 = ch * P
                    p1 = min(p0 + P, n) - 1
                    if p0 < n:
                        lo[ch] = min(lo[ch], b_c[p0])
                        hi[ch] = max(hi[ch], b_c[p1])
            # chunks with no edges on any core: aim at sb0
            for ch in range(k):
                if hi[ch] < 0:
                    lo[ch] = sb0
                    hi[ch] = sb0
            assert np.all(hi - lo <= 1), (hi - lo).max()

            # fill per-core arrays
            for c in range(N_CORES):
                starts, blk, rel, lidx, v = percore_sorted[c]
                s = starts[si * 4 + q]
                n = counts[c, si, q]
                for ch in range(k):
                    p0 = ch * P
                    p1 = min(p0 + P, n)
                    if p0 >= p1:
                        continue
                    g = t + ch
                    w = p1 - p0
                    bb = blk[s + p0 : s + p1]
                    rr = rel[s + p0 : s + p1]
                    assert np.all((bb == lo[ch]) | (bb == lo[ch] + 1))
                    idx_pads[c][g, :w] = lidx[s + p0 : s + p1]
                    val_pads[c][g, :w] = v[s + p0 : s + p1]
                    mlo = bb == lo[ch]
                    rlo_pads[c][g, :w] = np.where(mlo, rr, 200.0)
                    rhi_pads[c][g, :w] = np.where(~mlo, rr, 200.0)
            # split into calls
            for c0 in range(0, k, MAXC):
                nn = min(MAXC, k - c0)
                chunks = [
                    (t + c0 + j, int(lo[c0 + j]), int(hi[c0 + j]))
                    for j in range(nn)
                ]
                sb_calls.append((q, chunks))
            t += k
        calls.append(sb_calls)
    assert t == n_chunks
    per_core = [
        (idx_pads[c], rlo_pads[c], rhi_pads[c], val_pads[c])
        for c in range(N_CORES)
    ]
    return calls, n_chunks, per_core


def _build_kernel(sb_call_list, n_chunks):
    import concourse.bass as bass
    import concourse.bacc as bacc
    import concourse.tile as tile
    from concourse import mybir

    nc = bacc.Bacc(
        "TRN2",
        target_bir_lowering=False,
        debug=False,
        num_devices=N_CORES,
        num_swdge_queues=4,
    )
    f32 = mybir.dt.float32
    i16 = mybir.dt.int16

    xT = nc.dram_tensor("xT", [IN_F, SHARD], f32, kind="ExternalInput")
    Wt = nc.dram_tensor("W", [IN_F, OUT_F], f32, kind="ExternalInput")
    bf16 = mybir.dt.bfloat16
    iota_in = nc.dram_tensor("iota", [P, 4 * P], bf16, kind="ExternalInput")
    idx_in = nc.dram_tensor("idx", [P, 8 * n_chunks], i16, kind="ExternalInput")
    rlo_in = nc.dram_tensor("rlo", [P, n_chunks], bf16, kind="ExternalInput")
    rhi_in = nc.dram_tensor("rhi", [P, n_chunks], bf16, kind="ExternalInput")
    val_in = nc.dram_tensor("val", [P, n_chunks], bf16, kind="ExternalInput")
    out_d = nc.dram_tensor("out", [SHARD, OUT_F], f32, kind="ExternalOutput")

    h_c = nc.dram_tensor("h_c", [SHARD, OUT_F], bf16)
    h_full = nc.dram_tensor("h_full", [N_PAD, OUT_F], bf16, addr_space="Shared")

    with tile.TileContext(nc) as tc:
        # ---------------- phase 1: h_c = x_c @ W, AllGather ----------------
        with (
            tc.tile_pool(name="p1x", bufs=1) as p1x,
            tc.tile_pool(name="p1w", bufs=1) as p1w,
            tc.tile_pool(name="p1o", bufs=4) as p1o,
            tc.tile_pool(name="p1ps", bufs=4, space="PSUM") as p1ps,
        ):
            xt_t = p1x.tile([P, 2, SHARD], f32)  # two K-halves
            nc.sync.dma_start(
                xt_t[:], xT[:].rearrange("(k p) s -> p k s", p=P)
            )
            w_t = p1w.tile([P, 2, OUT_F], f32)
            nc.sync.dma_start(w_t[:], Wt[:].rearrange("(k p) f -> p k f", p=P))
            for t in range(N_BLOCKS):
                ps = p1ps.tile([P, OUT_F], f32, space="PSUM")
                for k in range(2):
                    nc.tensor.matmul(
                        ps[:],
                        lhsT=xt_t[:, k, t * P : (t + 1) * P],
                        rhs=w_t[:, k, :],
                        start=(k == 0),
                        stop=(k == 1),
                    )
                ho = p1o.tile([P, OUT_F], bf16)
                nc.scalar.copy(ho[:], ps[:])
                nc.sync.dma_start(out=h_c[t * P : (t + 1) * P, :], in_=ho[:])
            nc.gpsimd.collective_compute(
                "AllGather",
                mybir.AluOpType.bypass,
                replica_groups=[list(range(N_CORES))],
                ins=[h_c[:]],
                outs=[h_full[:]],
            )

        # ---------------- phase 2: gather + one-hot matmul segsum ---------
        with (
            tc.tile_pool(name="meta", bufs=1) as meta,
            tc.tile_pool(name="msgs", bufs=8) as msgs_p,
            tc.tile_pool(name="sel", bufs=40) as sel_p,
            tc.tile_pool(name="outs", bufs=8) as outs_p,
            tc.tile_pool(name="ps", bufs=8, space="PSUM") as ps_p,
        ):
            iota_t = meta.tile([P, 4, P], bf16)
            nc.sync.dma_start(
                iota_t[:], iota_in[:].rearrange("p (g d) -> p g d", g=4)
            )
            idx_t = meta.tile([P, 8 * n_chunks], i16)
            nc.sync.dma_start(idx_t[:], idx_in[:])
            rlo_t = meta.tile([P, n_chunks], bf16)
            nc.sync.dma_start(rlo_t[:], rlo_in[:])
            rhi_t = meta.tile([P, n_chunks], bf16)
            nc.sync.dma_start(rhi_t[:], rhi_in[:])
            val_t = meta.tile([P, n_chunks], bf16)
            nc.sync.dma_start(val_t[:], val_in[:])

            call_no = 0
            for sb_calls in sb_call_list:
                psum_of_block = {}
                for q, chunks in sb_calls:
                    ncall = len(chunks)
                    ni = ncall * P
                    c0 = chunks[0][0]
                    m = msgs_p.tile([P, ncall, OUT_F], bf16, tag="m", name="m")
                    nc.gpsimd.dma_gather(
                        m[:],
                        h_full[q * QUART : (q + 1) * QUART, :],
                        idx_t[:, c0 * 8 : (c0 + ncall) * 8],
                        ni,
                        ni,
                        OUT_F,
                        single_packet=False,
                        queue_num=call_no % 4,
                    )
                    call_no += 1
                    s_lo = {}
                    s_hi = {}
                    for g0 in range(0, ncall, 4):
                        g = min(4, ncall - g0)
                        s4l = sel_p.tile([P, 4, P], bf16, tag="s4", name="s4l")
                        nc.vector.tensor_tensor(
                            out=s4l[:, :g, :],
                            in0=iota_t[:, :g, :],
                            in1=rlo_t[:, c0 + g0 : c0 + g0 + g, None].to_broadcast([P, g, P]),
                            op=mybir.AluOpType.is_equal,
                        )
                        nc.vector.tensor_tensor(
                            out=s4l[:, :g, :],
                            in0=s4l[:, :g, :],
                            in1=val_t[:, c0 + g0 : c0 + g0 + g, None].to_broadcast([P, g, P]),
                            op=mybir.AluOpType.mult,
                        )
                        straddle = any(
                            chunks[g0 + jj][2] != chunks[g0 + jj][1]
                            for jj in range(g)
                        )
                        s4h = None
                        if straddle:
                            s4h = sel_p.tile([P, 4, P], bf16, tag="s4", name="s4h")
                            nc.vector.tensor_tensor(
                                out=s4h[:, :g, :],
                                in0=iota_t[:, :g, :],
                                in1=rhi_t[:, c0 + g0 : c0 + g0 + g, None].to_broadcast([P, g, P]),
                                op=mybir.AluOpType.is_equal,
                            )
                            nc.vector.tensor_tensor(
                                out=s4h[:, :g, :],
                                in0=s4h[:, :g, :],
                                in1=val_t[:, c0 + g0 : c0 + g0 + g, None].to_broadcast([P, g, P]),
                                op=mybir.AluOpType.mult,
                            )
                        for jj in range(g):
                            s_lo[g0 + jj] = (s4l, jj)
                            s_hi[g0 + jj] = (s4h, jj)
                    for j, (ch, blo, bhi) in enumerate(chunks):
                        targets = [(blo, s_lo)]
                        if bhi != blo:
                            targets.append((bhi, s_hi))
                        for b, stab in targets:
                            if b not in psum_of_block:
                                psum_of_block[b] = (
                                    ps_p.tile([P, OUT_F], f32, space="PSUM", tag="psb", name="psb"),
                                    [],
                                )
                            ps, plist = psum_of_block[b]
                            s4, jj = stab[j]
                            plist.append((s4, jj, m, j))
                for b, (ps, plist) in sorted(psum_of_block.items()):
                    for n, (s4, jj, mm, j) in enumerate(plist):
                        nc.tensor.matmul(
                            ps[:],
                            lhsT=s4[:, jj, :],
                            rhs=mm[:, j, :],
                            start=(n == 0),
                            stop=(n == len(plist) - 1),
                        )
                    ot = outs_p.tile([P, OUT_F], f32)
                    nc.scalar.copy(ot[:], ps[:])
                    nc.sync.dma_start(
                        out=out_d[b * P : (b + 1) * P, :], in_=ot[:]
                    )

    nc.compile()
    return nc


def kernel(x, weight, edge_row, edge_col, edge_vals):
    x = np.ascontiguousarray(x, dtype=np.float32)
    weight = np.ascontiguousarray(weight, dtype=np.float32)
    edge_row = np.asarray(edge_row, dtype=np.int64)
    edge_col = np.asarray(edge_col, dtype=np.int64)
    edge_vals = np.asarray(edge_vals, dtype=np.float32)

    trace = os.environ.get("GCN_TRACE", "0") == "1"
    if trace:
        _enable_profiling()

    from concourse.bass_utils import run_bass_kernel_spmd

    sb_call_list, n_chunks, per_core = _prep(edge_row, edge_col, edge_vals)
    nc = _build_kernel(sb_call_list, n_chunks)

    # host-side input staging
    x_pad = np.zeros((N_PAD, IN_F), dtype=np.float32)
    x_pad[:N_NODES] = x
    import ml_dtypes

    iota = np.broadcast_to(
        np.tile(np.arange(P, dtype=np.float32), 4)[None, :], (P, 4 * P)
    ).astype(ml_dtypes.bfloat16)

    in_maps = []
    flat_calls = [cq for sb_calls in sb_call_list for cq in sb_calls]
    for c in range(N_CORES):
        idx_pad, rlo_pad, rhi_pad, val_pad = per_core[c]
        # wrap indices per call: within a call of n chunks (= n*128 idxs),
        # idx i -> partition i%16, col i//16, replicated to 128 partitions.
        idx_w = np.zeros((P, 8 * n_chunks), dtype=np.int16)
        for q, chunks in flat_calls:
            c0 = chunks[0][0]
            n = len(chunks)
            flat = idx_pad[c0 : c0 + n].reshape(n * P)
            w = flat.reshape(n * 8, 16).T  # [16, n*8]
            idx_w[:, c0 * 8 : (c0 + n) * 8] = np.tile(w, (8, 1))
        in_maps.append(
            {
                "xT": np.ascontiguousarray(
                    x_pad[c * SHARD : (c + 1) * SHARD].T
                ),
                "W": weight,
                "iota": iota,
                "idx": idx_w,
                "rlo": np.ascontiguousarray(rlo_pad.T).astype(
                    ml_dtypes.bfloat16
                ),
                "rhi": np.ascontiguousarray(rhi_pad.T).astype(
                    ml_dtypes.bfloat16
                ),
                "val": np.ascontiguousarray(val_pad.T).astype(
                    ml_dtypes.bfloat16
                ),
            }
        )

    res = run_bass_kernel_spmd(
        nc,
        in_maps,
        list(range(N_CORES)),
        trace=trace,
        trace_cores=[0] if trace else None,
    )
    global LAST_EXEC_TIME_NS, LAST_RESULT
    LAST_EXEC_TIME_NS = res.exec_time_ns
    LAST_RESULT = res

    out = np.empty((N_NODES, OUT_F), dtype=np.float32)
    for c in range(N_CORES):
        lo = c * SHARD
        hi = min((c + 1) * SHARD, N_NODES)
        if hi > lo:
            out[lo:hi] = res.results[c]["out"][: hi - lo]
    return out
